# revision 5
# baseline (speedup 1.0000x reference)
"""DeltaNet (scalar-gated linear attention) Trainium2 Bass kernel.

Problem: B=2, L=2048, D=1024, H=8 heads, Dh=128.
  q,k,v = x @ {Wq,Wk,Wv}.T (split into heads), RoPE on q,k,
  alpha = sigmoid(x@Walpha.T), beta = 0.85 + 0.149*sigmoid(x@Wbeta.T + bbeta)
  recurrence over L: S_t = beta_t*S_{t-1} + alpha_t * k_t v_t^T ; o_t = S_t^T q_t
  out = o @ Wo.T + bo

Sharding: 16 (batch, head) pairs over 8 cores; core c handles batch c//4,
heads (2*(c%4), 2*(c%4)+1).  Scan is chunked (C=128) so everything becomes
128x128 matmuls; within-chunk decay is folded as:
  o_t = P_t * [ q_t^T S0 + sum_{s<=t} (alpha_s/P_s) (k_s.q_t) v_s ]
  S_C = P_C*S0 + sum_s (P_C*alpha_s/P_s) k_s v_s^T
with P_t = prod_{r<=t} beta_r (within chunk).  Each core returns a partial
y (its 2 heads through Wo); host sums groups of 4 cores and adds bo.
"""
import sys

sys.path.insert(0, "/opt/trn_rl_repo")

import numpy as np

import concourse.bass as bass
import concourse.mybir as mybir
import concourse.tile as tile
from concourse import bacc
from concourse.bass_utils import run_bass_kernel_spmd

F32 = mybir.dt.float32
F32R = mybir.dt.float32r
MUL = mybir.AluOpType.mult
ADD = mybir.AluOpType.add
BYP = mybir.AluOpType.bypass
COPY = mybir.ActivationFunctionType.Copy
SIGM = mybir.ActivationFunctionType.Sigmoid

B, L, D, H = 2, 2048, 1024, 8
DH = 128          # head dim
C = 128           # chunk length
NCH = L // C      # 16 chunks per sequence
KC = D // 128     # 8 contraction chunks
NHL = 2           # heads per core
BETA_MIN, BETA_MAX = 0.85, 0.999

_NC_CACHE = {}


def _build():
    nc = bacc.Bacc("TRN2", target_bir_lowering=False, debug=False)

    # ---- DRAM I/O (per-core views; same NEFF on all 8 cores) ----
    xT_d = nc.dram_tensor("xT", [D, L], F32R, kind="ExternalInput")
    wq_d = nc.dram_tensor("wq", [D, NHL * DH], F32R, kind="ExternalInput")
    wk_d = nc.dram_tensor("wk", [D, NHL * DH], F32R, kind="ExternalInput")
    wv_d = nc.dram_tensor("wv", [D, NHL * DH], F32R, kind="ExternalInput")
    wab_d = nc.dram_tensor("wab", [D, 4], F32R, kind="ExternalInput")
    wo_d = nc.dram_tensor("wo", [NHL * DH, D], F32R, kind="ExternalInput")
    cos2_d = nc.dram_tensor("cos2", [128, L], F32, kind="ExternalInput")
    sin2n_d = nc.dram_tensor("sin2n", [128, L], F32, kind="ExternalInput")
    caus_d = nc.dram_tensor("caus", [128, 128], F32, kind="ExternalInput")
    ident_d = nc.dram_tensor("ident", [128, 128], F32, kind="ExternalInput")
    biasa_d = nc.dram_tensor("biasa", [2 * NCH, 1], F32, kind="ExternalInput")
    biasb_d = nc.dram_tensor("biasb", [2 * NCH, 1], F32, kind="ExternalInput")
    y_d = nc.dram_tensor("y", [L, D], F32, kind="ExternalOutput")

    with tile.TileContext(nc) as tc:
        with (
            tc.tile_pool(name="sb", bufs=1) as sb,
            tc.tile_pool(name="pp", bufs=1, space="PSUM") as pp,
        ):
            # ---------- load constants / weights ----------
            xts = sb.tile([128, KC, L], F32R, tag="xT")
            nc.sync.dma_start(xts[:], xT_d.rearrange("(c p) n -> p c n", p=128))
            wqs = sb.tile([128, KC, NHL * DH], F32R, tag="wq")
            nc.sync.dma_start(wqs[:], wq_d.rearrange("(c p) m -> p c m", p=128))
            wks = sb.tile([128, KC, NHL * DH], F32R, tag="wk")
            nc.sync.dma_start(wks[:], wk_d.rearrange("(c p) m -> p c m", p=128))
            wvs = sb.tile([128, KC, NHL * DH], F32R, tag="wv")
            nc.sync.dma_start(wvs[:], wv_d.rearrange("(c p) m -> p c m", p=128))
            wabs = sb.tile([128, KC, 4], F32R, tag="wab")
            nc.sync.dma_start(wabs[:], wab_d.rearrange("(c p) m -> p c m", p=128))
            wos = sb.tile([128, NHL, D], F32R, tag="wo")
            nc.sync.dma_start(wos[:], wo_d.rearrange("(c p) m -> p c m", p=128))
            cos2s = sb.tile([128, L], F32, tag="cos2")
            nc.sync.dma_start(cos2s[:], cos2_d[:])
            sin2ns = sb.tile([128, L], F32, tag="sin2n")
            nc.sync.dma_start(sin2ns[:], sin2n_d[:])
            causs = sb.tile([128, 128], F32, tag="caus")
            nc.sync.dma_start(causs[:], caus_d[:])
            idents = sb.tile([128, 128], F32, tag="ident")
            nc.sync.dma_start(idents[:], ident_d[:])
            biasa_s = sb.tile([2 * NCH, 1], F32, tag="biasa")
            nc.sync.dma_start(biasa_s[:], biasa_d[:])
            biasb_s = sb.tile([2 * NCH, 1], F32, tag="biasb")
            nc.sync.dma_start(biasb_s[:], biasb_d[:])

            c1_32 = sb.tile([32, 128], F32, tag="c1")
            nc.vector.memset(c1_32[:], 1.0)
            c2_32 = sb.tile([32, 128], F32, tag="c2")
            nc.vector.memset(c2_32[:], 2.0)
            cbm_32 = sb.tile([32, 128], F32, tag="cbm")
            nc.vector.memset(cbm_32[:], BETA_MIN)

            # ---------- projections ----------
            qh = [sb.tile([128, L], F32, tag=f"q{h}", name=f"q{h}") for h in range(NHL)]
            kh = [sb.tile([128, L], F32, tag=f"k{h}", name=f"k{h}") for h in range(NHL)]
            vpos = sb.tile([128, NCH, NHL * DH], F32, tag="vpos")
            gca = sb.tile([2 * NCH, 128], F32, tag="gca")  # alpha rows (h0 0-15, h1 16-31)
            gcb = sb.tile([2 * NCH, 128], F32, tag="gcb")  # beta rows

            NT = L // 512  # 4 n-tiles for q/k/gates
            for dst, wsrc in ((qh, wqs), (kh, wks)):
                for m in range(NHL):
                    for nt in range(NT):
                        pq = pp.tile([128, 512], F32, tag="big", bufs=2, name="pq")
                        for kc in range(KC):
                            nc.tensor.matmul(
                                pq[:],
                                wsrc[:, kc, m * DH:(m + 1) * DH],
                                xts[:, kc, nt * 512:(nt + 1) * 512],
                                start=(kc == 0), stop=(kc == KC - 1),
                            )
                        nc.scalar.copy(dst[m][:, nt * 512:(nt + 1) * 512], pq[:])

            # gates: logits [4, 512] per n-tile -> scatter to chunk-major
            for nt in range(NT):
                pg = pp.tile([4, 512], F32, tag="big", bufs=2, name="pg")
                for kc in range(KC):
                    nc.tensor.matmul(
                        pg[:], wabs[:, kc, :],
                        xts[:, kc, nt * 512:(nt + 1) * 512],
                        start=(kc == 0), stop=(kc == KC - 1),
                    )
                gsb = sb.tile([4, 512], F32, tag="gsb", bufs=2, name="gsb")
                nc.scalar.copy(gsb[:], pg[:])
                for g, (dst, roff) in enumerate(
                    ((gca, 0), (gca, NCH), (gcb, 0), (gcb, NCH))
                ):
                    nc.sync.dma_start(
                        dst[roff + nt * 4: roff + nt * 4 + 4, :],
                        gsb[g:g + 1, :].rearrange("one (c s) -> one c s", c=4),
                    )

            # v in position-major [t, e]: lhsT = xT chunk, rhs = wv
            for ci in range(NCH):
                pv = pp.tile([128, NHL * DH], F32, tag="med", bufs=2, name="pv")
                for kc in range(KC):
                    nc.tensor.matmul(
                        pv[:], xts[:, kc, ci * 128:(ci + 1) * 128],
                        wvs[:, kc, :],
                        start=(kc == 0), stop=(kc == KC - 1),
                    )
                nc.scalar.copy(vpos[:, ci, :], pv[:])

            # ---------- RoPE on q, k (feature-major, block layout) ----------
            # r = q*cos2 + swap(q)*sin2n  (in place; swap via SBUF->SBUF DMA)
            for ti, t in enumerate(qh + kh):
                for j in range(4):
                    sl = slice(j * 512, (j + 1) * 512)
                    tsw = sb.tile([128, 512], F32, tag="tsw", bufs=2, name="tsw")
                    nc.sync.dma_start(tsw[0:64, :], t[64:128, sl])
                    nc.sync.dma_start(tsw[64:128, :], t[0:64, sl])
                    eng1 = nc.vector if (ti + j) % 2 == 0 else nc.gpsimd
                    eng2 = nc.gpsimd if (ti + j) % 2 == 0 else nc.vector
                    eng1.scalar_tensor_tensor(t[:, sl], t[:, sl], 1.0,
                                              cos2s[:, sl], MUL, MUL)
                    eng2.scalar_tensor_tensor(tsw[:], tsw[:], 1.0,
                                              sin2ns[:, sl], MUL, MUL)
                    nc.vector.scalar_tensor_tensor(t[:, sl], t[:, sl], 1.0,
                                                   tsw[:], MUL, ADD)

            # ---------- gate pipeline ----------
            # sigmoid with per-row bias
            nc.scalar.activation(gca[:], gca[:], SIGM, bias=biasa_s[:])
            nc.scalar.activation(gcb[:], gcb[:], SIGM, bias=biasb_s[:])
            # beta = BETA_MIN + (BETA_MAX-BETA_MIN)*sig
            nc.vector.scalar_tensor_tensor(
                gcb[:], gcb[:], BETA_MAX - BETA_MIN, cbm_32[:], MUL, ADD)
            # P = within-chunk cumprod of beta
            Pa = sb.tile([2 * NCH, 128], F32, tag="Pa")
            nc.vector.tensor_tensor_scan(Pa[:], gcb[:], gcb[:], 1.0, MUL, BYP)
            # rP = 1/P with one extra full-precision Newton step
            rPa = sb.tile([2 * NCH, 128], F32, tag="rPa")
            nc.vector.reciprocal(rPa[:], Pa[:])
            nrt = sb.tile([2 * NCH, 128], F32, tag="nrt")
            nc.vector.scalar_tensor_tensor(nrt[:], Pa[:], -1.0, rPa[:], MUL, MUL)
            nc.vector.scalar_tensor_tensor(nrt[:], nrt[:], 1.0, c2_32[:], MUL, ADD)
            nc.vector.scalar_tensor_tensor(rPa[:], nrt[:], 1.0, rPa[:], MUL, MUL)
            # g = alpha/P ; g2 = g * P_end
            ga = sb.tile([2 * NCH, 128], F32, tag="ga")
            nc.vector.scalar_tensor_tensor(ga[:], gca[:], 1.0, rPa[:], MUL, MUL)
            g2a = sb.tile([2 * NCH, 128], F32, tag="g2a")
            nc.vector.scalar_tensor_tensor(g2a[:], ga[:], Pa[:, 127:128], c1_32[:], MUL, MUL)
            # column forms via PE transpose
            gaT = sb.tile([128, 2 * NCH], F32, tag="gaT")
            g2aT = sb.tile([128, 2 * NCH], F32, tag="g2aT")
            for src, dst in ((ga, gaT), (g2a, g2aT)):
                ptr = pp.tile([128, 2 * NCH], F32, tag="sm1", bufs=2, name="ptr")
                nc.tensor.transpose(ptr[:], src[:], idents[0:32, 0:32])
                nc.scalar.copy(dst[:], ptr[:])

            # ---------- chunked scan + output projection ----------
            oh = [sb.tile([128, L], F32R, tag=f"o{h}", name=f"o{h}") for h in range(NHL)]
            S = [[sb.tile([128, DH], F32, tag=f"S{h}_{p}", name=f"S{h}_{p}")
                  for p in range(2)] for h in range(NHL)]
            for h in range(NHL):
                for p in range(2):
                    nc.vector.memset(S[h][p][:], 0.0)

            for ci in range(NCH):
                for h in range(NHL):
                    idx = h * NCH + ci
                    qs = qh[h][:, ci * 128:(ci + 1) * 128]
                    ks = kh[h][:, ci * 128:(ci + 1) * 128]
                    vs = vpos[:, ci, h * DH:(h + 1) * DH]
                    Scur = S[h][ci % 2]
                    Snxt = S[h][(ci + 1) % 2]

                    # broadcast P row / P_end for this chunk
                    pbc = sb.tile([128, 128], F32, tag="pbc", bufs=3, name="pbc")
                    nc.gpsimd.partition_broadcast(pbc[:], Pa[idx:idx + 1, :])
                    pend = sb.tile([128, 1], F32, tag="pend", bufs=3, name="pend")
                    nc.gpsimd.partition_broadcast(pend[:], Pa[idx:idx + 1, 127:128])

                    work = pp.tile([128, 256], F32, tag="med", bufs=2, name="work")
                    # k chunk -> position-major, scaled by g2 (ACT copy w/ scale)
                    nc.tensor.transpose(work[:, 128:256], ks, idents[:])
                    ktil = sb.tile([128, 128], F32, tag="ktil", bufs=3, name="ktil")
                    nc.scalar.activation(ktil[:], work[:, 128:256], COPY,
                                         scale=g2aT[:, idx:idx + 1])
                    # Gram G[s,t] = k_s . q_t ; A = G * g_s * causal
                    nc.tensor.matmul(work[:, 0:128], ks, qs, start=True, stop=True)
                    Asb = sb.tile([128, 128], F32, tag="Asb", bufs=3, name="Asb")
                    nc.vector.scalar_tensor_tensor(
                        Asb[:], work[:, 0:128], gaT[:, idx:idx + 1], causs[:], MUL, MUL)

                    # O^T[e,t] = v^T A + S0^T q ; oT = O^T * P_t
                    ops = pp.tile([128, 128], F32, tag="sm1", bufs=2, name="ops")
                    nc.tensor.matmul(ops[:], vs, Asb[:], start=True, stop=False)
                    nc.tensor.matmul(ops[:], Scur[:], qs, start=False, stop=True)
                    nc.vector.scalar_tensor_tensor(
                        oh[h][:, ci * 128:(ci + 1) * 128], ops[:], 1.0, pbc[:], MUL, MUL)

                    # state update: S' = P_end*S + ktil^T v
                    ups = pp.tile([128, DH], F32, tag="sm2", bufs=2, name="ups")
                    nc.tensor.matmul(ups[:], ktil[:], vs, start=True, stop=True)
                    nc.vector.scalar_tensor_tensor(
                        Snxt[:], Scur[:], pend[:], ups[:], MUL, ADD)

                # output projection for each finished 512-block
                if ci % 4 == 3:
                    for tt in range(ci - 3, ci + 1):
                        for ncol in range(2):
                            yps = pp.tile([128, 512], F32, tag="big", bufs=2, name="yps")
                            for h in range(NHL):
                                nc.tensor.matmul(
                                    yps[:], oh[h][:, tt * 128:(tt + 1) * 128],
                                    wos[:, h, ncol * 512:(ncol + 1) * 512],
                                    start=(h == 0), stop=(h == NHL - 1),
                                )
                            ysb = sb.tile([128, 512], F32, tag="ysb",
                                          bufs=2, name="ysb")
                            if (tt + ncol) % 2 == 0:
                                nc.scalar.copy(ysb[:], yps[:])
                            else:
                                nc.vector.tensor_copy(ysb[:], yps[:])
                            nc.sync.dma_start(
                                y_d[tt * 128:(tt + 1) * 128,
                                    ncol * 512:(ncol + 1) * 512], ysb[:])

    nc.compile()
    return nc


def _get_nc():
    if "nc" not in _NC_CACHE:
        _NC_CACHE["nc"] = _build()
    return _NC_CACHE["nc"]


def _rope_tables():
    half = DH // 2
    inv_freq = 1.0 / (10000.0 ** (np.arange(half, dtype=np.float64) / half))
    freqs = np.arange(L, dtype=np.float64)[:, None] * inv_freq[None, :]  # [L, 64]
    cosF = np.cos(freqs).T.astype(np.float32)   # [64, L]
    sinF = np.sin(freqs).T.astype(np.float32)
    cos2 = np.concatenate([cosF, cosF], axis=0)          # [128, L]
    sin2n = np.concatenate([-sinF, sinF], axis=0)        # [128, L]
    return cos2, sin2n


def kernel(x, Wq, Wk, Wv, Wbeta, bbeta, Walpha, balpha, Wo, bo):
    x = np.asarray(x, np.float32)
    Wq, Wk, Wv, Wo = (np.asarray(a, np.float32) for a in (Wq, Wk, Wv, Wo))
    Wbeta, Walpha = np.asarray(Wbeta, np.float32), np.asarray(Walpha, np.float32)
    bbeta, balpha = np.asarray(bbeta, np.float32), np.asarray(balpha, np.float32)
    bo = np.asarray(bo, np.float32)

    nc = _get_nc()
    perm = np.concatenate([np.arange(0, DH, 2), np.arange(1, DH, 2)])
    cos2, sin2n = _rope_tables()
    caus = np.triu(np.ones((128, 128), dtype=np.float32))
    ident = np.eye(128, dtype=np.float32)

    xT = [np.ascontiguousarray(x[b].T) for b in range(B)]

    in_maps = []
    for c in range(8):
        b = c // 4
        h1, h2 = 2 * (c % 4), 2 * (c % 4) + 1
        wq_c = np.concatenate(
            [Wq[h * DH:(h + 1) * DH][perm].T for h in (h1, h2)], axis=1)
        wk_c = np.concatenate(
            [Wk[h * DH:(h + 1) * DH][perm].T for h in (h1, h2)], axis=1)
        wv_c = np.concatenate(
            [Wv[h * DH:(h + 1) * DH].T for h in (h1, h2)], axis=1)
        wab_c = np.stack(
            [Walpha[h1], Walpha[h2], Wbeta[h1], Wbeta[h2]], axis=1)
        wo_c = np.concatenate(
            [Wo[:, h * DH:(h + 1) * DH].T for h in (h1, h2)], axis=0)
        biasa = np.repeat([balpha[h1], balpha[h2]], NCH).astype(np.float32)[:, None]
        biasb = np.repeat([bbeta[h1], bbeta[h2]], NCH).astype(np.float32)[:, None]
        in_maps.append({
            "xT": xT[b], "wq": np.ascontiguousarray(wq_c),
            "wk": np.ascontiguousarray(wk_c), "wv": np.ascontiguousarray(wv_c),
            "wab": np.ascontiguousarray(wab_c), "wo": np.ascontiguousarray(wo_c),
            "cos2": cos2, "sin2n": sin2n, "caus": caus, "ident": ident,
            "biasa": biasa, "biasb": biasb,
        })

    res = run_bass_kernel_spmd(nc, in_maps, core_ids=list(range(8)))
    y = np.zeros((B, L, D), dtype=np.float32)
    for c in range(8):
        y[c // 4] += res.results[c]["y"]
    y += bo
    return y
